# revision 1
# baseline (speedup 1.0000x reference)
"""Self-contained Trainium2 kernel for nn_FDN_37211596653125.

kernel(**inputs) -> y (32,2,441000) float32.
Host: FDN impulse response (tiny 6x6 solves). Device (8 NeuronCores, raw
Bass): overlap-save FFT convolution, N=131072=256x512 Cooley-Tukey via PE
matmuls, twiddle/spectral products on VectorE, PSUM evacuation on ScalarE.
"""
import sys
sys.path.insert(0, "/opt/trn_rl_repo")
import numpy as np
import concourse.bass as bass
import concourse.mybir as mybir
from concourse.masks import make_identity


SR = 44100
IR_LEN = 44100
T60 = 0.75
D = 6


def _expm(A):
    """Pade-13 scaling-and-squaring matrix exponential (float64)."""
    A = A.astype(np.float64)
    b = [64764752532480000.0, 32382376266240000.0, 7771770303897600.0,
         1187353796428800.0, 129060195264000.0, 10559470521600.0,
         670442572800.0, 33522128640.0, 1323241920.0, 40840800.0,
         960960.0, 16380.0, 182.0, 1.0]
    n = A.shape[0]
    nrm = np.linalg.norm(A, 1)
    theta13 = 5.371920351148152
    s = max(0, int(np.ceil(np.log2(max(nrm / theta13, 1e-300)))))
    if nrm <= theta13:
        s = 0
    A = A / (2.0 ** s)
    I = np.eye(n)
    A2 = A @ A
    A4 = A2 @ A2
    A6 = A2 @ A4
    U = A @ (A6 @ (b[13] * A6 + b[11] * A4 + b[9] * A2)
             + b[7] * A6 + b[5] * A4 + b[3] * A2 + b[1] * I)
    V = (A6 @ (b[12] * A6 + b[10] * A4 + b[8] * A2)
         + b[6] * A6 + b[4] * A4 + b[2] * A2 + b[0] * I)
    R = np.linalg.solve(V - U, V + U)
    for _ in range(s):
        R = R @ R
    return R


def fdn_ir(b, c, U_raw, gamma_raw, delays):
    """Build (2,2,IR_LEN) float32 FDN impulse response."""
    delays_f = delays.astype(np.float32)
    F_ = IR_LEN // 2 + 1
    gamma_max = np.float32(10.0) ** (np.float32(-60.0 / SR / T60 / 20.0)
                                     * delays_f)               # (D,)
    gamma = (1.0 / (1.0 + np.exp(-gamma_raw.astype(np.float32)))) * gamma_max  # (S,D)
    S = gamma.shape[0]
    pos = np.arange(F_, dtype=np.float32) * np.float32((S - 1) / (F_ - 1))
    lo = np.clip(np.floor(pos).astype(np.int32), 0, S - 2)
    frac = (pos - lo.astype(np.float32))[:, None]
    g = gamma[lo] * (1.0 - frac) + gamma[lo + 1] * frac         # (F,D) fp32

    tri = np.triu(U_raw.astype(np.float64), 1)
    U = _expm(tri - tri.T).astype(np.float32)                   # (D,D)

    A = U[None, :, :] * g[:, None, :]                           # (F,D,D)
    freqs = (np.arange(F_, dtype=np.float32) / np.float32(IR_LEN)
             * np.float32(2.0 * np.pi))
    phase = freqs[:, None] * delays_f[None, :]                  # fp32 like ref
    invD = np.exp(1j * phase.astype(np.float32)).astype(np.complex64)  # (F,D)
    eye = np.eye(D, dtype=np.complex64)
    M = invD[:, :, None] * eye[None] - A.astype(np.complex64)   # (F,D,D)
    b_c = np.broadcast_to(b.astype(np.complex64), (F_, D, 2))
    X = np.linalg.solve(M, b_c)                                 # (F,D,2)
    H = np.einsum('oi,fik->fok', c.astype(np.complex64), X)     # (F,2,2)
    h = np.fft.irfft(H.transpose(1, 2, 0), n=IR_LEN, axis=-1)   # (2,2,L)
    return h.astype(np.float32)





FP = mybir.dt.float32

N = 131072
N1, N2 = 256, 512
K_IR = 44100
L_HOP = N - K_IR + 1            # 86973
T_SIG = 441000
NBLK = 6
PAD_PRE = K_IR - 1              # 44099
PAD_LEN = (NBLK - 1) * L_HOP + N  # 565937
NB = 4                          # batches per core
NIT = NB * NBLK                 # 24

# ---- per-iteration semaphore increment schedules -------------------------
PE_PER = 40   # A0=1, T0=8, C0=4, A1=1, T1=8, C1=4, Ap=4, Tp=8, Cp=2
DVE_PER = 5   # tw0, fdl0, tw1, fdl1, twp
ACT_PER = 36  # evA0=2, cT0=8, evX0=2, evA1=2, cT1=8, evX1=2, evAp=2, cS=8, cy=2


def pe_m(it):
    b = PE_PER * it
    return dict(A0=b + 1, T0=[b + 1 + k for k in range(1, 9)],
                C0=[b + 9 + k for k in range(1, 5)], A1=b + 14,
                T1=[b + 14 + k for k in range(1, 9)],
                C1=[b + 22 + k for k in range(1, 5)],
                Ap=[b + 26 + k for k in range(1, 5)],
                Tp=[b + 30 + k for k in range(1, 9)],
                Cp=[b + 38 + k for k in range(1, 3)])


def dve_m(it):
    b = DVE_PER * it
    return dict(tw0=b + 1, fdl0=b + 2, tw1=b + 3, fdl1=b + 4, twp=b + 5)


def act_m(it):
    b = ACT_PER * it
    return dict(evA0=[b + 1, b + 2],
                cT0=[b + 2 + k for k in range(1, 9)],
                evX0=[b + 11, b + 12],
                evA1=[b + 13, b + 14],
                cT1=[b + 14 + k for k in range(1, 9)],
                evX1=[b + 23, b + 24],
                evAp=[b + 25, b + 26],
                cS=[b + 26 + k for k in range(1, 9)],
                cy=[b + 35, b + 36])


def out_ranges(j):
    """DMA row ranges for valid region of block j."""
    valid = min(L_HOP, T_SIG - j * L_HOP)
    p0, p1 = PAD_PRE, PAD_PRE + valid
    res = []
    for n1c in range(2):
        base = 128 * n1c
        segs = []
        for R in range(base, base + 128):
            r0, r1 = 512 * R, 512 * R + 512
            s, e = max(r0, p0), min(r1, p1)
            if s >= e:
                continue
            segs.append((R, s - r0, e - r0, s))
        i = 0
        while i < len(segs):
            R, c0, c1, dst = segs[i]
            if c0 == 0 and c1 == 512:
                k = i
                while (k + 1 < len(segs) and segs[k + 1][1] == 0
                       and segs[k + 1][2] == 512):
                    k += 1
                res.append((n1c, segs[i][0] - base, segs[k][0] - base + 1,
                            0, 512, dst - p0 + j * L_HOP))
                i = k + 1
            else:
                res.append((n1c, R - base, R - base + 1, c0, c1,
                            dst - p0 + j * L_HOP))
                i += 1
    return res


def n_out_dmas(j):
    return 2 * len(out_ranges(j))   # x2 planes


BF = mybir.dt.bfloat16


def _sched():
    """Counter values for the stage-interleaved software-pipelined schedule.
    PE slot s: A0(s) Ap(s-1) T0(s) C0(s) Tp(s-1) A1(s) Cp(s-1) T1(s) C1(s)
    ACT slot s: evA0(s) evAp(s-1) cT0(s) evX0(s) cS(s-1) evA1(s) cy(s-1)
                cT1(s) evX1(s)
    DVE slot s: tw0(s) twp(s-1) fdl0(s) tw1(s) fdl1(s)"""
    PEV, DVEV, ACTV, GPV = {}, {}, {}, {}
    pe = dve = act = 0
    gp = 1   # make_identity inc
    for s in range(NIT + 1):
        f = s < NIT
        v = s >= 1
        it = s - 1
        if f:
            pe += 1; PEV[(s, "A0")] = pe
        if v:
            PEV[(it, "Ap")] = [pe + k for k in range(1, 5)]; pe += 4
        if f:
            PEV[(s, "T0")] = [pe + k for k in range(1, 9)]; pe += 8
            PEV[(s, "C0")] = [pe + k for k in range(1, 5)]; pe += 4
        if v:
            PEV[(it, "Tp")] = [pe + k for k in range(1, 9)]; pe += 8
        if f:
            pe += 1; PEV[(s, "A1")] = pe
        if v:
            PEV[(it, "Cp")] = [pe + k for k in range(1, 3)]; pe += 2
        if f:
            PEV[(s, "T1")] = [pe + k for k in range(1, 9)]; pe += 8
            PEV[(s, "C1")] = [pe + k for k in range(1, 5)]; pe += 4
        if f:
            dve += 1; DVEV[(s, "tw0")] = dve
        if v:
            DVEV[(it, "evAp")] = []
            DVEV[(it, "twp")] = []
            for _h in range(2):
                DVEV[(it, "evAp")] += [dve + 1, dve + 2]; dve += 2
                dve += 1; DVEV[(it, "twp")] += [dve]
        if f:
            dve += 1; DVEV[(s, "fdl0")] = dve
            dve += 1; DVEV[(s, "tw1")] = dve
        if f:
            DVEV[(s, "fdl1")] = [dve + k for k in range(1, 5)]; dve += 4
        if f:
            ACTV[(s, "evA0")] = [act + k for k in range(1, 5)]; act += 4
            ACTV[(s, "cT0")] = [act + k for k in range(1, 5)]; act += 4

        if f:
            ACTV[(s, "evX0")] = [act + k for k in range(1, 5)]; act += 4
        if v:
            ACTV[(it, "cS")] = [act + k for k in range(1, 9)]; act += 8
        if f:
            ACTV[(s, "evA1")] = [act + 1, act + 2]; act += 2
        if v:
            ACTV[(it, "cy")] = [act + k for k in range(1, 5)]; act += 4
        if f:
            ACTV[(s, "cT1")] = [act + k for k in range(1, 5)]; act += 4
            ACTV[(s, "evX1")] = [act + k for k in range(1, 5)]; act += 4
    return PEV, DVEV, ACTV, GPV


PEV, DVEV, ACTV, GPV = _sched()
BF = mybir.dt.bfloat16


def build_nc():
    nc = bass.Bass()
    FR = mybir.dt.float32r
    xp_in = nc.declare_dram_parameter("xp", [NB, 2, PAD_LEN], FR, isOutput=False)
    w256_in = nc.declare_dram_parameter("w256", [128, 12 * 128], FR, isOutput=False)
    w256h_in = nc.declare_dram_parameter("w256h", [128, 12 * 128], BF, isOutput=False)
    w512_in = nc.declare_dram_parameter("w512", [128, 48 * 128], BF, isOutput=False)
    tw_in = nc.declare_dram_parameter("tw", [128, 4 * N2], BF, isOutput=False)
    twi_in = nc.declare_dram_parameter("twi", [128, 8 * 256], BF, isOutput=False)
    g_in = nc.declare_dram_parameter("g", [128, 16 * 256], BF, isOutput=False)
    y_out = nc.declare_dram_parameter("y", [NB, 2, T_SIG], FP, isOutput=True)

    NTAB = 6
    ld_after_ch0 = {}
    ld_after = {}
    out_after = {}
    v = 16 * NTAB
    for it in range(NIT):
        v += 32; ld_after_ch0[it] = v
        v += 32; ld_after[it] = v
        if it >= 1:
            v += 16 * n_out_dmas((it - 1) % NBLK); out_after[it - 1] = v
    v += 16 * n_out_dmas((NIT - 1) % NBLK); out_after[NIT - 1] = v

    from contextlib import ExitStack
    es = ExitStack()
    with es:
        w256 = es.enter_context(nc.sbuf_tensor([128, 12 * 128],
                                                mybir.dt.float32r))
        w256h = es.enter_context(nc.sbuf_tensor([128, 12 * 128], BF))
        w512 = es.enter_context(nc.sbuf_tensor([128, 48 * 128], BF))
        tw = es.enter_context(nc.sbuf_tensor([128, 4 * N2], BF))
        twi = es.enter_context(nc.sbuf_tensor([128, 8 * 256], BF))
        gtab = es.enter_context(nc.sbuf_tensor([128, 16 * 256], BF))
        ident = es.enter_context(nc.sbuf_tensor([128, 128], FP))
        identb = es.enter_context(nc.sbuf_tensor([128, 128], BF))
        m1 = es.enter_context(nc.sbuf_tensor([128, 2 * 2048],
                                              mybir.dt.float32r))
        absb0 = es.enter_context(nc.sbuf_tensor([128, 2048], BF))
        absb1 = es.enter_context(nc.sbuf_tensor([128, 2048], BF))
        xsb0 = es.enter_context(nc.sbuf_tensor([128, 2048], BF))
        xsb1 = es.enter_context(nc.sbuf_tensor([128, 2048], BF))
        ssb = es.enter_context(nc.sbuf_tensor([128, 2048], BF))
        t2 = es.enter_context(nc.sbuf_tensor([128, 2048], BF))
        t2t = es.enter_context(nc.sbuf_tensor([128, 2048], BF))
        ymatA = es.enter_context(nc.sbuf_tensor([128, 2048], BF))
        ymatB = es.enter_context(nc.sbuf_tensor([128, 2048], BF))
        s2 = es.enter_context(nc.sbuf_tensor([128, 2048], BF))
        s2t = es.enter_context(nc.sbuf_tensor([128, 2048], BF))
        ysb = es.enter_context(nc.sbuf_tensor([128, 2 * 2048], FP))
        dvetmp = es.enter_context(nc.sbuf_tensor([128, 1024], BF))
        pa = es.enter_context(nc.psum_tensor("pa", [128, 2048], FP))
        pcs = [es.enter_context(nc.psum_tensor(f"pc{i}", [128, 512], FP))
               for i in range(4)]
        s_dma = es.enter_context(nc.semaphore("s_dma"))
        s_pe = es.enter_context(nc.semaphore("s_pe"))
        s_dve = es.enter_context(nc.semaphore("s_dve"))
        s_act = es.enter_context(nc.semaphore("s_act"))
        s_gp = es.enter_context(nc.semaphore("s_gp"))
        block = es.enter_context(nc.Block())

        def ymat_(it):
            return ymatA if it % 2 == 0 else ymatB

        def psT(p):
            t = pcs[p // 4]             # 8 slots in pc0/pc1 (bf16)
            o = (p % 4) * 256
            return t[:, :].bitcast(BF)[:, o:o + 256]

        def w256c(n1c, k1c, plane):
            o = (plane * 4 + n1c * 2 + k1c) * 128
            return w256[:, o:o + 128]

        def w256hc(k1c, n1c, plane):
            o = (plane * 4 + k1c * 2 + n1c) * 128
            return w256h[:, o:o + 128]

        def w512c(n2c, k2c, plane):
            o = (plane * 16 + n2c * 4 + k2c) * 128
            return w512[:, o:o + 128]

        def t2c(k1c, plane):
            return t2[:, (plane * 2 + k1c) * N2:(plane * 2 + k1c + 1) * N2]

        def t2tc(n2c, plane):
            return t2t[:, (plane * 4 + n2c) * 256:(plane * 4 + n2c + 1) * 256]

        def ymc(it, k2c, plane):
            y = ymat_(it)
            return y[:, (plane * 4 + k2c) * 256:(plane * 4 + k2c + 1) * 256]

        def s2c(n2c, plane):
            return s2[:, (plane * 4 + n2c) * 256:(plane * 4 + n2c + 1) * 256]

        def s2tc(k1c, plane):
            return s2t[:, (plane * 2 + k1c) * N2:(plane * 2 + k1c + 1) * N2]

        R32 = mybir.dt.float32r

        def mm_r(dst, w, x, **kw):
            return nc.tensor.matmul(dst, w.bitcast(R32), x.bitcast(R32), **kw)

        def cmul6(eng, dre, dim, sre, sim_, wre, wim, t1, accum=False):
            if not accum:
                eng.tensor_mul(dre, sre, wre)
                eng.tensor_mul(t1, sim_, wim)
                eng.tensor_sub(dre, dre, t1)
                eng.tensor_mul(dim, sre, wim)
                eng.tensor_mul(t1, sim_, wre)
                return eng.tensor_add(dim, dim, t1)
            eng.tensor_mul(t1, sre, wre)
            eng.tensor_add(dre, dre, t1)
            eng.tensor_mul(t1, sim_, wim)
            eng.tensor_sub(dre, dre, t1)
            eng.tensor_mul(t1, sre, wim)
            eng.tensor_add(dim, dim, t1)
            eng.tensor_mul(t1, sim_, wre)
            return eng.tensor_add(dim, dim, t1)

        @block.gpsimd
        def _(gpsimd):
            make_identity(nc, ident[:, :])
            make_identity(nc, identb[:, :])
            gpsimd.sem_inc(s_gp, 1)

        def emit_outs(sync, it):
            b, j = divmod(it, NBLK)
            buf = it % 2
            sync.wait_ge(s_act, ACTV[(it, "cy")][3])
            for plane in range(2):
                for (n1c, rlo, rhi, clo, chi, doff) in out_ranges(j):
                    o_ = buf * 2048 + (plane * 2 + n1c) * N2
                    src = ysb[rlo:rhi, o_ + clo:o_ + chi]
                    cnt = (rhi - rlo) * (chi - clo)
                    dst = y_out[b, plane, doff:doff + cnt]
                    dst = dst.rearrange("(p f) -> p f", f=chi - clo)
                    sync.dma_start(dst, src).then_inc(s_dma, 16)

        @block.sync
        def _(sync):
            sync.dma_start(w256[:, :], w256_in[:, :]).then_inc(s_dma, 16)
            sync.dma_start(w256h[:, :], w256h_in[:, :]).then_inc(s_dma, 16)
            sync.dma_start(w512[:, :], w512_in[:, :]).then_inc(s_dma, 16)
            sync.dma_start(tw[:, :], tw_in[:, :]).then_inc(s_dma, 16)
            sync.dma_start(twi[:, :], twi_in[:, :]).then_inc(s_dma, 16)
            sync.dma_start(gtab[:, :], g_in[:, :]).then_inc(s_dma, 16)
            for it in range(NIT):
                b, j = divmod(it, NBLK)
                buf = it % 2
                if it >= 2:
                    sync.wait_ge(s_pe, PEV[(it - 2, "A1")])
                for ch in range(2):
                    src = xp_in[b, ch, j * L_HOP: j * L_HOP + N]
                    src = src.rearrange("(p f) -> p f", f=N2)
                    for n1c in range(2):
                        o_ = buf * 2048 + (ch * 2 + n1c) * N2
                        sync.dma_start(
                            m1[:, o_:o_ + N2],
                            src[128 * n1c:128 * (n1c + 1), :],
                        ).then_inc(s_dma, 16)
                if it >= 1:
                    emit_outs(sync, it - 1)
            emit_outs(sync, NIT - 1)

        @block.tensor
        def _(tensor):
            tensor.wait_ge(s_gp, 1)
            tensor.wait_ge(s_dma, 16 * 6)

            def m1c(s, ch, n1c):
                o_ = (s % 2) * 2048 + (ch * 2 + n1c) * N2
                return m1[:, o_:o_ + N2]

            def stageA(s, ch):
                last = None
                for k1c in range(2):
                    pre = pa[:, 512 * k1c:512 * (k1c + 1)]
                    pim = pa[:, 1024 + 512 * k1c:1024 + 512 * (k1c + 1)]
                    for n1c in range(2):
                        mm_r(pre, w256c(n1c, k1c, 0), m1c(s, ch, n1c),
                             start=(n1c == 0), stop=(n1c == 1))
                    for n1c in range(2):
                        last = mm_r(pim, w256c(n1c, k1c, 1), m1c(s, ch, n1c),
                                    start=(n1c == 0), stop=(n1c == 1))
                last.then_inc(s_pe, 1)

            def transposes(cTk):
                p = 0
                for n2c in range(4):
                    for k1c in range(2):
                        slot = psT(p)
                        nc.tensor.transpose(
                            slot[:, 0:128],
                            t2c(k1c, 0)[:, n2c * 128:(n2c + 1) * 128],
                            identb[:, :])
                        nc.tensor.transpose(
                            slot[:, 128:256],
                            t2c(k1c, 1)[:, n2c * 128:(n2c + 1) * 128],
                            identb[:, :]).then_inc(s_pe, 1)
                        p += 1

            def transposesP(cSk, twpv):
                p = 0
                for n2c in range(4):
                    for k1c in range(2):
                        if p == 4:
                            tensor.wait_ge(s_dve, twpv[1])
                        slot = psT(p)
                        nc.tensor.transpose(
                            slot[:, 0:128],
                            s2c(n2c, 0)[:, k1c * 128:(k1c + 1) * 128],
                            identb[:, :])
                        nc.tensor.transpose(
                            slot[:, 128:256],
                            s2c(n2c, 1)[:, k1c * 128:(k1c + 1) * 128],
                            identb[:, :]).then_inc(s_pe, 1)
                        p += 1

            def stageC(s, cTk):
                for gi, k2c in enumerate([2, 3, 0, 1]):
                    if k2c == 0:
                        tensor.wait_ge(s_act, cTk[2])
                    elif k2c == 1:
                        tensor.wait_ge(s_act, cTk[3])
                    pre = pcs[k2c][:, 0:256]
                    pim = pcs[k2c][:, 256:512]
                    seq = []
                    for n2c in range(4):
                        seq.append((pre, w512c(n2c, k2c, 0), t2tc(n2c, 0)))
                    for n2c in range(4):
                        seq.append((pre, w512c(n2c, k2c, 2), t2tc(n2c, 1)))
                    for i_, (dst, w_, r_) in enumerate(seq):
                        if gi == 0 and i_ == 2:
                            tensor.wait_ge(s_act, cTk[1])
                        if gi == 0 and i_ == 4:
                            tensor.wait_ge(s_act, cTk[3])
                        nc.tensor.matmul(dst, w_, r_, start=(i_ == 0),
                                         stop=(i_ == 7))
                    seq = []
                    for n2c in range(4):
                        seq.append((pim, w512c(n2c, k2c, 0), t2tc(n2c, 1)))
                    for n2c in range(4):
                        seq.append((pim, w512c(n2c, k2c, 1), t2tc(n2c, 0)))
                    for i_, (dst, w_, r_) in enumerate(seq):
                        mmv = nc.tensor.matmul(dst, w_, r_, start=(i_ == 0),
                                               stop=(i_ == 7))
                    mmv.then_inc(s_pe, 1)

            for s in range(NIT + 1):
                f, v = s < NIT, s >= 1
                it = s - 1
                # ---- A0(s) ----
                if f:
                    tensor.wait_ge(s_dma, ld_after_ch0[s])
                    if s == 1:
                        tensor.wait_ge(s_act, ACTV[(0, "evA1")][1])
                    elif s >= 2:
                        tensor.wait_ge(s_dve, DVEV[(s - 2, "evAp")][3])
                    stageA(s, 0)
                # ---- Ap(it) : inverse stage A' ----
                if v:
                    tensor.wait_ge(s_dve, DVEV[(it, "fdl1")][0])
                    if f:
                        tensor.wait_ge(s_act, ACTV[(s, "evA0")][1])
                    else:
                        tensor.wait_ge(s_act, ACTV[(it, "evA1")][1])
                    # half0 of pa (n2c 0,1) is free after evA0[1]
                    for n2c in range(4):
                        if n2c == 2 and f:
                            tensor.wait_ge(s_act, ACTV[(s, "evA0")][3])
                        pre = pa[:, 256 * n2c:256 * (n2c + 1)]
                        pim = pa[:, 1024 + 256 * n2c:1024 + 256 * (n2c + 1)]
                        seq = []
                        for k2c in [2, 3, 0, 1]:
                            seq.append((pre, w512c(k2c, n2c, 0),
                                        ymc(it, k2c, 0)))
                        for k2c in [2, 3, 0, 1]:
                            seq.append((pre, w512c(k2c, n2c, 1),
                                        ymc(it, k2c, 1)))
                        for i_, (dst, w_, r_) in enumerate(seq):
                            if n2c == 0 and 1 <= i_ <= 3:
                                tensor.wait_ge(s_dve,
                                               DVEV[(it, "fdl1")][i_])
                            nc.tensor.matmul(dst, w_, r_, start=(i_ == 0),
                                             stop=(i_ == 7))
                        seq = []
                        for k2c in range(4):
                            seq.append((pim, w512c(k2c, n2c, 0),
                                        ymc(it, k2c, 1)))
                        for k2c in range(4):
                            seq.append((pim, w512c(k2c, n2c, 2),
                                        ymc(it, k2c, 0)))
                        for i_, (dst, w_, r_) in enumerate(seq):
                            mmv = nc.tensor.matmul(dst, w_, r_,
                                                   start=(i_ == 0),
                                                   stop=(i_ == 7))
                        mmv.then_inc(s_pe, 1)
                # ---- T0(s) + C0(s) ----
                if f:
                    tensor.wait_ge(s_dve, DVEV[(s, "tw0")])
                    if s >= 1:
                        tensor.wait_ge(s_act, ACTV[(s - 1, "evX1")][3])
                    transposes(ACTV[(s, "cT0")])
                    tensor.wait_ge(s_act, ACTV[(s, "cT0")][0])
                    stageC(s, ACTV[(s, "cT0")])
                # ---- Tp(it) ----
                if v:
                    tensor.wait_ge(s_dve, DVEV[(it, "twp")][0])
                    if f:
                        tensor.wait_ge(s_act, ACTV[(s, "evX0")][3])
                    else:
                        tensor.wait_ge(s_act, ACTV[(it, "evX1")][3])
                    transposesP(ACTV[(it, "cS")], DVEV[(it, "twp")])
                # ---- A1(s) ----
                if f:
                    tensor.wait_ge(s_dma, ld_after[s])
                    if s >= 1:
                        tensor.wait_ge(s_dve, DVEV[(s - 1, "evAp")][3])
                    else:
                        tensor.wait_ge(s_act, ACTV[(0, "evA0")][1])
                    stageA(s, 1)
                # ---- Cp(it) : inverse stage C' ----
                if v:
                    tensor.wait_ge(s_act, ACTV[(it, "cS")][7])
                    if f:
                        tensor.wait_ge(s_act, ACTV[(s, "evX0")][3])
                    for n1c in range(2):
                        pre = pcs[n1c][:, :]
                        pim = pcs[2 + n1c][:, :]
                        seq = []
                        for k1c in range(2):
                            seq.append((pre, w256hc(k1c, n1c, 0),
                                        s2tc(k1c, 0)))
                        for k1c in range(2):
                            seq.append((pre, w256hc(k1c, n1c, 1),
                                        s2tc(k1c, 1)))
                        for i_, (dst, w_, r_) in enumerate(seq):
                            nc.tensor.matmul(dst[:, :], w_, r_,
                                             start=(i_ == 0), stop=(i_ == 3))
                        seq = []
                        for k1c in range(2):
                            seq.append((pim, w256hc(k1c, n1c, 0),
                                        s2tc(k1c, 1)))
                        for k1c in range(2):
                            seq.append((pim, w256hc(k1c, n1c, 2),
                                        s2tc(k1c, 0)))
                        for i_, (dst, w_, r_) in enumerate(seq):
                            mmv = nc.tensor.matmul(dst[:, :], w_, r_,
                                                   start=(i_ == 0),
                                                   stop=(i_ == 3))
                        mmv.then_inc(s_pe, 1)
                # ---- T1(s) + C1(s) ----
                if f:
                    tensor.wait_ge(s_dve, DVEV[(s, "tw1")])
                    if s >= 1:
                        tensor.wait_ge(s_act, ACTV[(s - 1, "cS")][7])
                        tensor.wait_ge(s_act, ACTV[(s - 1, "cy")][1])
                    transposes(ACTV[(s, "cT1")])
                    tensor.wait_ge(s_act, ACTV[(s, "cT1")][0])
                    if s >= 1:
                        tensor.wait_ge(s_act, ACTV[(s - 1, "cy")][3])
                    stageC(s, ACTV[(s, "cT1")])

        @block.vector
        def _(vector):
            t1 = dvetmp[:, 0:1024]
            for s in range(NIT + 1):
                f, v = s < NIT, s >= 1
                it = s - 1
                if f:
                    vector.wait_ge(s_act, ACTV[(s, "evA0")][3])
                    cmul6(nc.vector, t2[:, 0:1024], t2[:, 1024:2048],
                          absb0[:, 0:1024], absb0[:, 1024:2048],
                          tw[:, 0:1024], tw[:, 1024:2048], t1
                          ).then_inc(s_dve, 1)
                if v:
                    for h in range(2):
                        o = h * 512
                        vector.wait_ge(s_pe, PEV[(it, "Ap")][2 * h + 1])
                        nc.vector.tensor_copy(
                            ssb[:, o:o + 512],
                            pa[:, o:o + 512]).then_inc(s_dve, 1)
                        nc.vector.tensor_copy(
                            ssb[:, 1024 + o:1536 + o],
                            pa[:, 1024 + o:1536 + o]).then_inc(s_dve, 1)
                        cmul6(nc.vector, s2[:, o:o + 512],
                              s2[:, 1024 + o:1536 + o],
                              ssb[:, o:o + 512], ssb[:, 1024 + o:1536 + o],
                              twi[:, o:o + 512], twi[:, 1024 + o:1536 + o],
                              t1[:, 0:512]).then_inc(s_dve, 1)
                if f:
                    ym = ymat_(s)
                    vector.wait_ge(s_act, ACTV[(s, "evX0")][3])
                    x3 = xsb0[:, :].rearrange("p (a b) -> p a b", b=512)
                    cmul6(nc.vector,
                          ym[:, 0:1024].rearrange("p (a b) -> p a b", b=256),
                          ym[:, 1024:2048].rearrange("p (a b) -> p a b", b=256),
                          x3[:, :, 0:256], x3[:, :, 256:512],
                          gtab[:, 0:1024].rearrange("p (a b) -> p a b", b=256),
                          gtab[:, 1024:2048].rearrange("p (a b) -> p a b",
                                                       b=256),
                          t1[:, 0:1024].rearrange("p (a b) -> p a b", b=256)
                          ).then_inc(s_dve, 1)
                    vector.wait_ge(s_act, ACTV[(s, "evA1")][1])
                    cmul6(nc.vector, t2[:, 0:1024], t2[:, 1024:2048],
                          absb1[:, 0:1024], absb1[:, 1024:2048],
                          tw[:, 0:1024], tw[:, 1024:2048], t1
                          ).then_inc(s_dve, 1)
                if f:
                    ym = ymat_(s)
                    for gi, c4 in enumerate([2, 3, 0, 1]):
                        o = c4 * 256
                        vector.wait_ge(s_act, ACTV[(s, "evX1")][gi])
                        cmul6(nc.vector,
                              ym[:, o:o + 256], ym[:, 1024 + o:1280 + o],
                              xsb1[:, 2 * o:2 * o + 256],
                              xsb1[:, 2 * o + 256:2 * o + 512],
                              gtab[:, 2048 + o:2304 + o],
                              gtab[:, 3072 + o:3328 + o],
                              t1[:, 0:256], accum=True).then_inc(s_dve, 1)

        @block.scalar
        def _(scalar):
            def ev(src_t, dst, wait_pe, war_dve):
                scalar.wait_ge(s_pe, wait_pe)
                if war_dve is not None:
                    scalar.wait_ge(s_dve, war_dve)
                nc.scalar.copy(dst[:, 0:1024],
                               src_t[:, 0:1024]).then_inc(s_act, 1)
                nc.scalar.copy(dst[:, 1024:2048],
                               src_t[:, 1024:2048]).then_inc(s_act, 1)

            def pairT(pev, cTv):
                # 8 transposed pairs -> t2t; re copies first so stage C can
                # start after cT[1]
                for pl in range(2):
                    for h in range(2):
                        scalar.wait_ge(s_pe, pev[3 + 4 * h])
                        sl = pcs[h][:, :].bitcast(BF).rearrange(
                            "p (a b) -> p a b", b=256)
                        nc.scalar.copy(
                            t2t[:, pl * 1024 + h * 512:
                                pl * 1024 + h * 512 + 512].rearrange(
                                "p (a b) -> p a b", b=128),
                            sl[:, :, pl * 128:pl * 128 + 128]
                            ).then_inc(s_act, 1)

            def pairS(pev):
                # slot p=(n2c,k1c): re/im tile -> s2t[(pl*2+k1c)*512 + n2c*128]
                # plane-major so C' can start after cS[3]
                for pl in range(2):
                    for k1c in range(2):
                        for h in range(2):
                            scalar.wait_ge(s_pe, pev[3 + 4 * h])
                            sl4 = pcs[h][:, :].bitcast(BF).rearrange(
                                "p (n q b) -> p n q b", q=4, b=128)
                            o = (pl * 2 + k1c) * 512 + h * 256
                            nc.scalar.copy(
                                s2t[:, o:o + 256].rearrange(
                                    "p (a b) -> p a b", b=128),
                                sl4[:, :, k1c * 2 + pl, :]).then_inc(s_act, 1)

            for s in range(NIT + 1):
                f, v = s < NIT, s >= 1
                it = s - 1
                if f:
                    scalar.wait_ge(s_pe, PEV[(s, "A0")])
                    if s >= 1:
                        scalar.wait_ge(s_dve, DVEV[(s - 1, "tw0")])
                    for h in range(2):
                        o = h * 512
                        nc.scalar.copy(absb0[:, o:o + 512],
                                       pa[:, o:o + 512]).then_inc(s_act, 1)
                        nc.scalar.copy(absb0[:, 1024 + o:1536 + o],
                                       pa[:, 1024 + o:1536 + o]
                                       ).then_inc(s_act, 1)
                    pairT(PEV[(s, "T0")], ACTV[(s, "cT0")])
                if f:
                    for gi, c4 in enumerate([2, 3, 0, 1]):
                        scalar.wait_ge(s_pe, PEV[(s, "C0")][gi])
                        if gi == 0 and s >= 1:
                            scalar.wait_ge(s_dve, DVEV[(s - 1, "fdl0")])
                        nc.scalar.copy(xsb0[:, c4 * 512:(c4 + 1) * 512],
                                       pcs[c4][:, :]).then_inc(s_act, 1)
                if v:
                    pairS(PEV[(it, "Tp")])
                if f:
                    ev(pa, absb1, PEV[(s, "A1")],
                       DVEV[(s - 1, "tw1")] if s >= 1 else None)
                if v:
                    if it >= 2:
                        scalar.wait_ge(s_dma, out_after[it - 2])
                    ob = (it % 2) * 2048
                    scalar.wait_ge(s_pe, PEV[(it, "Cp")][0])
                    nc.scalar.copy(ysb[:, ob:ob + 512],
                                   pcs[0][:, :]).then_inc(s_act, 1)
                    scalar.wait_ge(s_pe, PEV[(it, "Cp")][1])
                    nc.scalar.copy(ysb[:, ob + 512:ob + 1024],
                                   pcs[1][:, :]).then_inc(s_act, 1)
                    nc.scalar.copy(ysb[:, ob + 1024:ob + 1536],
                                   pcs[2][:, :]).then_inc(s_act, 1)
                    nc.scalar.copy(ysb[:, ob + 1536:ob + 2048],
                                   pcs[3][:, :]).then_inc(s_act, 1)
                if f:
                    pairT(PEV[(s, "T1")], ACTV[(s, "cT1")])
                    for gi, c4 in enumerate([2, 3, 0, 1]):
                        scalar.wait_ge(s_pe, PEV[(s, "C1")][gi])
                        if gi == 0 and s >= 1:
                            scalar.wait_ge(s_dve, DVEV[(s - 1, "fdl1")][3])
                        nc.scalar.copy(xsb1[:, c4 * 512:(c4 + 1) * 512],
                                       pcs[c4][:, :]).then_inc(s_act, 1)

    return nc


# ---------------------------- host side ----------------------------------

def make_device_tables(h):
    """h: (2,2,K_IR) float32 -> dict of DRAM table arrays (fp32)."""
    def dftm(n, sign):
        k = np.arange(n)
        return np.exp(sign * 2j * np.pi * np.outer(k, k) / n)
    W256 = dftm(N1, -1)
    W512 = dftm(N2, -1)
    k1 = np.arange(N1)
    n2 = np.arange(N2)
    TW = np.exp(-2j * np.pi * np.outer(k1, n2) / N)
    TWI = np.exp(+2j * np.pi * np.outer(n2, k1) / N) / N

    def chunks(M, pr, pc, planes):   # planes: list of 2d arrays [R,C]
        # returns [128, len(planes)*pr*pc*...] col-concatenated in
        # (plane, rowchunk, colchunk) order with 128-col chunks
        cols = []
        for P in planes:
            for a in range(pr):
                for b_ in range(pc):
                    cols.append(P[128 * a:128 * (a + 1),
                                  128 * b_:128 * (b_ + 1)].astype(np.float32))
        return np.ascontiguousarray(np.concatenate(cols, axis=1))

    w256 = chunks(None, 2, 2, [W256.real, W256.imag, -W256.imag])
    w512 = chunks(None, 4, 4, [W512.real, W512.imag, -W512.imag])

    twp = np.concatenate([TW.real[0:128], TW.real[128:256],
                          TW.imag[0:128], TW.imag[128:256]], axis=1)
    twip = np.concatenate([TWI.real[128 * a:128 * (a + 1)] for a in range(4)]
                          + [TWI.imag[128 * a:128 * (a + 1)] for a in range(4)],
                          axis=1)
    hp = np.zeros((2, 2, N), np.float64)
    hp[:, :, :K_IR] = h
    gcols = []
    for i in range(2):
        G = np.fft.fft(hp[0, i]) + 1j * np.fft.fft(hp[1, i])
        Gm = G.reshape(N2, N1)          # [k2, k1]
        for plane in range(2):
            P = Gm.real if plane == 0 else Gm.imag
            for k2c in range(4):
                gcols.append(P[128 * k2c:128 * (k2c + 1), :].astype(np.float32))
    g = np.ascontiguousarray(np.concatenate(gcols, axis=1))
    import ml_dtypes
    bf = ml_dtypes.bfloat16
    return dict(w256=w256.astype(np.float32), w256h=w256.astype(bf),
                w512=w512.astype(bf),
                tw=twp.astype(bf), twi=twip.astype(bf),
                g=g.astype(bf))


_NC_CACHE = None


def conv_device(x, h):
    """x: (B,2,T) fp32, h: (2,2,K_IR) fp32 -> y: (B,2,T) fp32 via 8 cores."""
    global _NC_CACHE
    from concourse.bass_utils import run_bass_kernel_spmd
    B = x.shape[0]
    assert B == 8 * NB
    xp = np.zeros((B, 2, PAD_LEN), np.float32)
    xp[:, :, PAD_PRE:PAD_PRE + T_SIG] = x
    tabs = make_device_tables(h)
    if _NC_CACHE is None:
        _NC_CACHE = build_nc()
    nc = _NC_CACHE
    in_maps = []
    for c in range(8):
        m = {"xp": xp[NB * c:NB * (c + 1)]}
        m.update(tabs)
        in_maps.append(m)
    res = run_bass_kernel_spmd(nc, in_maps, list(range(8)))
    y = np.concatenate([res.results[c]["y"] for c in range(8)], axis=0)
    return y




def kernel(**inputs):
    """Full FDN: build IR on host, FFT-convolve on 8 NeuronCores."""
    x = np.asarray(inputs["x"], np.float32)
    h = fdn_ir(np.asarray(inputs["b"]), np.asarray(inputs["c"]),
               np.asarray(inputs["U_raw"]), np.asarray(inputs["gamma_raw"]),
               np.asarray(inputs["delays"]))
    y = conv_device(x, h)
    return y.astype(np.float32)

